# revision 30
# baseline (speedup 1.0000x reference)
"""Trainium2 Bass kernel for nn_AbsorbanceLookup (retrieval_knn).

Contract: kernel(**inputs) -> np.ndarray. Takes the FULL unsharded inputs
(keys as in reference.setup_inputs()), distributes across 8 NeuronCores
(pure data parallel on the batch dim), returns the FULL [B] output.

Design (v3): everything lives in the natural query layout
nat[p, j] = query q = 256*p + j of the per-core slice (Bc = 32768).

  Hit path (DVE, bit-exact with the reference fp32 sequence, verified
  zero mask flips on the reference inputs): denormalize c/wl, nearest-grid
  via the 2^23 magic-round trick, fused abs-max tolerance compare, flat
  index g = c_idx*601 + wl_idx written directly as int16.

  Exact values: ONE gpsimd ap_gather per iteration (gchunks=1) from a
  per-partition replica of the host-prenormalized (A-mean)/std flat table
  (device-broadcast from a [1, 9616] input to keep host->device traffic
  small). Output is group-wrapped (core g serves partitions 16g..16g+15);
  only partitions ::16 are DMA'd out; host unwraps + merges (np.where).

  Interp path: the 2->128->256->128->1 tanh MLP is a smooth function of
  TWO scalars and is replaced by a total-degree-6 bivariate polynomial
  (28 terms) in u = clamp(x, +-6)/8, fit on host per-call from the weight
  inputs via fp64 lstsq (max |err| 1.8e-4 over [-6,6]^2 vs the exact MLP,
  ~100x under the rel-2e-2 gate; ~6e-5 on the reference distribution).
  Evaluated on DVE (power basis in u_w, per-degree FMAs with [P,1]
  runtime-coefficient APs iterated j-outer for pipelining, Horner in u_c)
  with the 7 independent G_i starter ops on ACT.

Measured engine characteristics that shaped this design:
  - ap_gather: ~115us for 32768 idxs (22-28ns/idx, Q7 software) -- THE
    bottleneck; everything else must hide under it.
  - DVE ops CONTEND ~1:1 with the concurrent gather (shared SBUF ports):
    every DVE-busy us adds ~1us of wall time; so the DVE op count is
    minimized (~44 ops) rather than the DVE critical path.
  - ACT is ~free under the gather for INDEPENDENT ops, but its exec queue
    has depth 0, so dependency-chained ACT ops pay full SBUF latency
    (~220 cycles) per hop -- chains stay on DVE.
  - PE identity-matmul psum accumulation is free under the gather but
    cross-engine ACT->PE->DVE poly pipelines measured SLOWER overall.
  - SWDGE dma_gather: ~8.6ns/idx, 1024-descriptor ring cap, Pool-serial
    -- strictly worse than ap_gather here.
  - qPoolDynamic indirect_dma_start consumes ONE offset per partition-row
    descriptor (embedding-row semantics), not per element -- unusable for
    per-element gathers.

HW exec time: ~118-124us/iteration loop-slope (baseline 181.3us), with
rel err 6.4e-5 (baseline 3.2e-5; gate 2e-2).
"""

import sys

if "/opt/trn_rl_repo" not in sys.path:
    sys.path.insert(0, "/opt/trn_rl_repo")

import numpy as np

import concourse.bass as bass
import concourse.tile as tile
from concourse import bacc, mybir
from concourse.ap import AP
from concourse.bass_utils import run_bass_kernel_spmd

F32 = mybir.dt.float32
I16 = mybir.dt.int16
U8 = mybir.dt.uint8
ALU = mybir.AluOpType

B = 262144
N_CORES = 8
BC = B // N_CORES          # 32768 per core
P = 128
FC = BC // P               # 256 free columns in natural layout
N_CONCS = 16
N_WL = 601
TBL = N_CONCS * N_WL       # 9616
MAGIC = 8388608.0          # 2^23: x + MAGIC - MAGIC == round-to-nearest-int(x)
C_MEAN, C_STD = 30.0, 30.0
WL_MEAN, WL_STD = 500.0, 300.0

DEG = 6                    # total degree of the interp polynomial
NTERMS = (DEG + 1) * (DEG + 2) // 2            # 28
USCALE = 0.125             # u = clamp(x, +-6) * 0.125  (exact pow2)
UCLAMP = 6.0 * USCALE


def coef_col(i, j):
    """Column of coefficient (i=c-degree, j=w-degree) in the s_coef tile."""
    c = 0
    for ii in range(i):
        c += DEG + 1 - ii
    return c + j


def build_nc(loop_n=1, dyn_loop=0, skip_hit=False, skip_poly=False,
             skip_gather=False, gchunks=1, gather_only=False, staggered=False):
    nc = bacc.Bacc("TRN2", target_bir_lowering=False, debug=False,
                   num_devices=N_CORES)

    # ---- dram I/O ----
    d_cn = nc.dram_tensor("cn", [P, FC], F32, kind="ExternalInput").ap()
    d_wn = nc.dram_tensor("wn", [P, FC], F32, kind="ExternalInput").ap()
    d_tbl1 = nc.dram_tensor("tbl1", [1, TBL], F32, kind="ExternalInput").ap()
    d_coef = nc.dram_tensor("coef", [P, NTERMS + 3], F32, kind="ExternalInput").ap()
    d_ident = nc.dram_tensor("ident", [P, P], F32, kind="ExternalInput").ap()
    d_out = nc.dram_tensor("out", [P, FC], F32, kind="ExternalOutput").ap()
    d_mh = nc.dram_tensor("maskh", [P, FC], U8, kind="ExternalOutput").ap()
    NJ = BC // 8                    # 4096 gather stream positions per group
    cj = NJ // gchunks
    d_exc = [
        nc.dram_tensor(f"exc{k}", [8, cj], F32, kind="ExternalOutput").ap()
        for k in range(gchunks)
    ]

    with tile.TileContext(nc) as tc:
        with (
            tc.tile_pool(name="const", bufs=1) as cpool,
            tc.tile_pool(name="hit", bufs=2) as hpool,
            tc.tile_pool(name="poly", bufs=2) as ppool,
            tc.tile_pool(name="gout", bufs=2) as gpool,
            tc.tile_pool(name="pep", bufs=1, space="PSUM") as pepool,
        ):
            s_ident = cpool.tile([P, P], F32, tag="ident")
            nc.sync.dma_start(s_ident[:], d_ident)
            s_cn = cpool.tile([P, FC], F32, tag="cn")
            nc.sync.dma_start(s_cn[:], d_cn)
            s_wn = cpool.tile([P, FC], F32, tag="wn")
            nc.sync.dma_start(s_wn[:], d_wn)
            s_coef = cpool.tile([P, NTERMS + 3], F32, tag="coef")
            nc.sync.dma_start(s_coef[:], d_coef)
            # broadcast the flat table to all 128 partitions (device side,
            # so the host->device input stays [1, TBL])
            s_tbl = cpool.tile([P, TBL], F32, tag="tbl")
            if not skip_gather:
                src = AP(d_tbl1.tensor, 0, [(0, P), (1, TBL)])
                nc.sync.dma_start(s_tbl[:], src)

            def coef(i, j):
                c = coef_col(i, j)
                return s_coef[:, c:c + 1]

            def _body():
                if gather_only:
                    g16o = hpool.tile([P, FC], I16, tag="g16o")
                    nc.vector.memset(g16o[:], 0)
                    cu = FC // gchunks
                    for k in range(gchunks):
                        dstk = gpool.tile([P, cj], F32, tag=f"dst{k}")
                        nc.gpsimd.ap_gather(
                            dstk[:], s_tbl[:], g16o[:, k * cu:(k + 1) * cu],
                            channels=P, num_elems=TBL, d=1, num_idxs=cj)
                        nc.sync.dma_start(d_exc[k], dstk[::16, :])
                    f0 = ppool.tile([P, FC], F32, tag="f")
                    nc.vector.memset(f0[0:8, 0:1], 0.0)
                    nc.sync.dma_start(d_out, f0[:])
                    nc.sync.dma_start(d_mh, g16o[:].bitcast(U8)[:, 0:FC])
                    return
                # ============== hit path (natural layout) ==============
                if not skip_hit:
                    cM = hpool.tile([P, FC], F32, tag="cM")
                    nc.vector.tensor_scalar(cM[:], s_cn[:], C_STD, C_MEAN,
                                            ALU.mult, ALU.add)
                    wM = hpool.tile([P, FC], F32, tag="wM")
                    nc.vector.tensor_scalar(wM[:], s_wn[:], WL_STD, WL_MEAN,
                                            ALU.mult, ALU.add)

                    # nearest conc index (x4): rc4 = 4*clip(round(c/4), 0, 15)
                    r1 = hpool.tile([P, FC], F32, tag="r1")
                    nc.vector.tensor_scalar(r1[:], cM[:], 0.25, MAGIC,
                                            ALU.mult, ALU.add)
                    rc = hpool.tile([P, FC], F32, tag="rc")
                    nc.vector.tensor_scalar(rc[:], r1[:], MAGIC, 0.0,
                                            ALU.subtract, ALU.max)
                    rc4 = hpool.tile([P, FC], F32, tag="rc4")
                    nc.vector.tensor_scalar(rc4[:], rc[:], 15.0, 4.0,
                                            ALU.min, ALU.mult)
                    dC = hpool.tile([P, FC], F32, tag="dC")
                    nc.vector.scalar_tensor_tensor(dC[:], rc4[:], -1.0, cM[:],
                                                   ALU.mult, ALU.add)

                    # nearest wavelength: rw6 = clip(round(wl), 200, 800) - 200
                    r1w = hpool.tile([P, FC], F32, tag="r1w")
                    nc.vector.tensor_scalar(r1w[:], wM[:], MAGIC, None, ALU.add)
                    rw = hpool.tile([P, FC], F32, tag="rw")
                    nc.vector.tensor_scalar(rw[:], r1w[:], MAGIC, 200.0,
                                            ALU.subtract, ALU.max)
                    rw6 = hpool.tile([P, FC], F32, tag="rw6")
                    nc.vector.tensor_scalar(rw6[:], rw[:], 800.0, 200.0,
                                            ALU.min, ALU.subtract)
                    ndW = hpool.tile([P, FC], F32, tag="ndW")
                    nc.vector.scalar_tensor_tensor(ndW[:], rw6[:], 200.0, wM[:],
                                                   ALU.add, ALU.subtract)

                    # flat gather index g = rc4*150.25 + rw6 (exact ints,
                    # i16 conversion fused into the op)
                    g16 = hpool.tile([P, FC], I16, tag="g16")
                    nc.vector.scalar_tensor_tensor(g16[:], rc4[:], 150.25,
                                                   rw6[:], ALU.mult, ALU.add)

                    # gather ASAP (Pool engine dominates the iteration)
                    cu = FC // gchunks
                    for k in range(gchunks):
                        dstk = gpool.tile([P, cj], F32, tag=f"dst{k}")
                        if not skip_gather:
                            nc.gpsimd.ap_gather(
                                dstk[:], s_tbl[:], g16[:, k * cu:(k + 1) * cu],
                                channels=P, num_elems=TBL, d=1, num_idxs=cj)
                        else:
                            nc.vector.memset(dstk[0:8, 0:1], 0.0)
                        nc.sync.dma_start(d_exc[k], dstk[::16, :])

                    # mask = max(|dC|, |ndW|) < 0.1  (exact: both-hit iff
                    # the max is under tol)
                    aC = hpool.tile([P, FC], F32, tag="aC")
                    nc.vector.scalar_tensor_tensor(aC[:], dC[:], -1.0, dC[:],
                                                   ALU.mult, ALU.max)
                    aW = hpool.tile([P, FC], F32, tag="aW")
                    nc.vector.scalar_tensor_tensor(aW[:], ndW[:], -1.0, ndW[:],
                                                   ALU.mult, ALU.max)
                    am = hpool.tile([P, FC], F32, tag="am")
                    nc.vector.tensor_tensor(am[:], aC[:], aW[:], ALU.max)
                    mask = hpool.tile([P, FC], U8, tag="mask")
                    nc.vector.tensor_scalar(mask[:], am[:], 0.1, None, ALU.is_lt)
                    nc.sync.dma_start(d_mh, mask[:])

                # ============== interp polynomial (DVE) ==============
                f = ppool.tile([P, FC], F32, tag="f")
                if skip_poly == "dummy_dve_psum":
                    # 64 dependent-ish DVE ops entirely in PSUM
                    pp = pepool.tile([P, 2 * FC], F32, tag="dps")
                    a0 = pp[:, 0:FC]
                    a1 = pp[:, FC:2 * FC]
                    nc.vector.memset(a0, 1.0001)
                    for z in range(64):
                        nc.vector.tensor_scalar(a1 if z % 2 == 0 else a0,
                                                a0 if z % 2 == 0 else a1,
                                                1.0001, None, ALU.mult)
                    nc.vector.memset(f[:], 0.0)
                elif skip_poly == "dummy_pe":
                    # 64 f32 identity matmuls into psum under the gather
                    ps = pepool.tile([P, FC], F32, tag="pep")
                    for z in range(64):
                        nc.tensor.matmul(ps[:], s_ident[:], s_cn[:],
                                         start=(z == 0), stop=(z == 63))
                    nc.vector.memset(f[:], 0.0)
                elif skip_poly == "dummy_act2":
                    dts = []
                    for z in range(4):
                        dmt = ppool.tile([P, FC], F32, tag=f"dm{z}")
                        dts.append(dmt)
                    for z in range(128):
                        nc.scalar.mul(dts[z % 4][:], s_cn[:], 1.0001)
                    nc.vector.memset(f[:], 0.0)
                elif skip_poly == "dummy_act":
                    dts = []
                    for z in range(4):
                        dmt = ppool.tile([P, FC], F32, tag=f"dm{z}")
                        dts.append(dmt)
                    for z in range(64):
                        nc.scalar.mul(dts[z % 4][:], s_cn[:], 1.0001)
                    nc.vector.memset(f[:], 0.0)
                elif skip_poly == "dummy":
                    # pipeline-friendly independent DVE ops, same count as
                    # the real poly, no deps on hit/gather tiles
                    dts = []
                    for z in range(4):
                        dmt = ppool.tile([P, FC], F32, tag=f"dm{z}")
                        dts.append(dmt)
                    for z in range(64):
                        nc.vector.tensor_scalar(dts[z % 4][:], s_cn[:],
                                                1.0001, None, ALU.mult)
                    nc.vector.memset(f[:], 0.0)
                elif skip_poly:
                    nc.vector.memset(f[:], 0.0)
                else:
                    ACTF = mybir.ActivationFunctionType
                    b6 = s_coef[:, NTERMS:NTERMS + 1]
                    b12 = s_coef[:, NTERMS + 1:NTERMS + 2]
                    buc = s_coef[:, NTERMS + 2:NTERMS + 3]

                    # clamp+scale on DVE (2 ops/dim)
                    uc = ppool.tile([P, FC], F32, tag="uc")
                    nc.vector.tensor_scalar(uc[:], s_cn[:], USCALE, UCLAMP,
                                            ALU.mult, ALU.min)
                    nc.vector.tensor_scalar(uc[:], uc[:], -UCLAMP, None, ALU.max)
                    uw = ppool.tile([P, FC], F32, tag="uw")
                    nc.vector.tensor_scalar(uw[:], s_wn[:], USCALE, UCLAMP,
                                            ALU.mult, ALU.min)
                    nc.vector.tensor_scalar(uw[:], uw[:], -UCLAMP, None, ALU.max)

                    # power basis in u_w on DVE
                    wpow = {1: uw}
                    for j in range(2, DEG + 1):
                        t = ppool.tile([P, FC], F32, tag=f"w{j}")
                        nc.vector.tensor_tensor(t[:], wpow[j - 1][:], uw[:],
                                                ALU.mult)
                        wpow[j] = t

                    # G_i starters on ACT (independent leaf ops, ~free
                    # under the gather); FMA accumulation on DVE, iterated
                    # j-outer so consecutive DVE ops touch different G_i
                    # (pipeline-friendly)
                    G = []
                    for i in range(DEG + 1):
                        Li = DEG - i
                        g = ppool.tile([P, FC], F32, tag=f"G{i}")
                        if Li == 0:
                            nc.scalar.activation(g[:], uw[:], ACTF.Identity,
                                                 bias=coef(i, 0), scale=0.0)
                        else:
                            nc.scalar.activation(g[:], uw[:], ACTF.Identity,
                                                 bias=coef(i, 0),
                                                 scale=coef(i, 1))
                        G.append(g)
                    for j in range(2, DEG + 1):
                        for i in range(0, DEG + 1 - j):
                            nc.vector.scalar_tensor_tensor(
                                G[i][:], wpow[j][:], coef(i, j), G[i][:],
                                ALU.mult, ALU.add)

                    # Horner over u_c on DVE
                    t1 = ppool.tile([P, FC], F32, tag="ht")
                    cur = G[DEG]
                    for i in range(DEG - 1, -1, -1):
                        nc.vector.tensor_tensor(t1[:], cur[:], uc[:], ALU.mult)
                        nc.vector.tensor_tensor(f[:], t1[:], G[i][:], ALU.add)
                        cur = f
                nc.sync.dma_start(d_out, f[:])

            if dyn_loop:
                with tc.For_i(0, dyn_loop, 1, staggered_reset=staggered):
                    _body()
            else:
                for _rep in range(loop_n):
                    _body()

    nc.finalize()
    return nc


_NC_CACHE = {}


def _get_nc():
    if "nc" not in _NC_CACHE:
        _NC_CACHE["nc"] = build_nc()
    return _NC_CACHE["nc"]


def fit_poly(w1, b1, w2, b2, w3, b3, w4, b4):
    """Fit the total-degree-DEG bivariate polynomial to the MLP composite
    over [-6, 6]^2 in the scaled variable u = x * USCALE. Returns [NTERMS]
    float32 coefficients (b4 folded into the constant term)."""
    w1, b1 = np.float64(w1), np.float64(b1)
    w2, b2 = np.float64(w2), np.float64(b2)
    w3, b3 = np.float64(w3), np.float64(b3)
    w4, b4 = np.float64(w4), np.float64(b4)

    ng = 48
    k = np.arange(ng)
    xg = np.cos(np.pi * (k + 0.5) / ng) * 6.0
    CG, WG = np.meshgrid(xg, xg, indexing="ij")
    x = np.stack([CG.ravel(), WG.ravel()], -1)
    h = np.tanh(x @ w1 + b1)
    h = np.tanh(h @ w2 + b2)
    h = np.tanh(h @ w3 + b3)
    fg = (h @ w4)[:, 0] + b4[0]

    uc = CG.ravel() * USCALE
    uw = WG.ravel() * USCALE
    cols = []
    for i in range(DEG + 1):
        for j in range(DEG + 1 - i):
            cols.append((uc ** i) * (uw ** j))
    A = np.stack(cols, -1)
    coefs, *_ = np.linalg.lstsq(A, fg, rcond=None)
    return coefs.astype(np.float32)


def make_in_maps(c_norm, wl_norm, train_concs, train_wavelengths, abs_matrix,
                 w1, b1, w2, b2, w3, b3, w4, b4):
    f32 = np.float32
    A = np.asarray(abs_matrix, f32)
    m = A.mean(dtype=f32).astype(f32)
    s = A.std(dtype=f32).astype(f32)
    tbl1 = ((A.ravel() - m) / s).astype(f32).reshape(1, TBL)

    coefs = fit_poly(w1, b1, w2, b2, w3, b3, w4, b4)
    coefs = np.concatenate([coefs, np.array([6.0, 12.0, UCLAMP], np.float32)])
    coef_t = np.ascontiguousarray(np.broadcast_to(coefs, (P, NTERMS + 3)))

    c_norm = np.asarray(c_norm, f32)
    wl_norm = np.asarray(wl_norm, f32)

    in_maps = []
    for i in range(N_CORES):
        sl = slice(i * BC, (i + 1) * BC)
        in_maps.append({
            "cn": c_norm[sl].reshape(P, FC),
            "wn": wl_norm[sl].reshape(P, FC),
            "tbl1": tbl1,
            "coef": coef_t,
            "ident": np.eye(P, dtype=f32),
        })
    return in_maps


def kernel(**inputs):
    nc = _get_nc()
    in_maps = make_in_maps(**inputs)
    res = run_bass_kernel_spmd(nc, in_maps, core_ids=list(range(N_CORES)))
    parts = []
    for i in range(N_CORES):
        r = res.results[i]
        interp = r["out"].astype(np.float32)                      # [P, FC]
        mask = r["maskh"] != 0                                    # [P, FC]
        # unwrap gather chunks: core-group g owns partitions 16g+r;
        # dstk[16g, 16*(j - j0) + r] = tbl[g16[16g + r, j]]
        V = np.empty((P, FC), np.float32)
        gch = len([k for k in r if k.startswith("exc")])
        cu = FC // gch
        for k in range(gch):
            E = r[f"exc{k}"].reshape(8, cu, 16)                   # [g, jc, r]
            V[:, k * cu:(k + 1) * cu] = E.transpose(0, 2, 1).reshape(P, cu)
        parts.append(np.where(mask, V, interp).ravel())
    return np.concatenate(parts).astype(np.float32)


# revision 32
# speedup vs baseline: 1.0085x; 1.0085x over previous
"""Trainium2 Bass kernel for nn_AbsorbanceLookup (retrieval_knn).

Contract: kernel(**inputs) -> np.ndarray. Takes the FULL unsharded inputs
(keys as in reference.setup_inputs()), distributes across 8 NeuronCores
(pure data parallel on the batch dim), returns the FULL [B] output.

Design (v3): everything lives in the natural query layout
nat[p, j] = query q = 256*p + j of the per-core slice (Bc = 32768).

  Hit path (DVE, bit-exact with the reference fp32 sequence, verified
  zero mask flips on the reference inputs): denormalize c/wl, nearest-grid
  via the 2^23 magic-round trick, fused abs-max tolerance compare, flat
  index g = c_idx*601 + wl_idx written directly as int16.

  Exact values: ONE gpsimd ap_gather per iteration (gchunks=1) from a
  per-partition replica of the host-prenormalized (A-mean)/std flat table
  (device-broadcast from a [1, 9616] input to keep host->device traffic
  small). Output is group-wrapped (core g serves partitions 16g..16g+15);
  only partitions ::16 are DMA'd out; host unwraps + merges (np.where).

  Interp path: the 2->128->256->128->1 tanh MLP is a smooth function of
  TWO scalars and is replaced by a total-degree-6 bivariate polynomial
  (28 terms) in u = clamp(x, +-6)/8, fit on host per-call from the weight
  inputs via fp64 lstsq (max |err| 1.8e-4 over [-6,6]^2 vs the exact MLP,
  ~100x under the rel-2e-2 gate; ~6e-5 on the reference distribution).
  Evaluated on DVE (power basis in u_w, per-degree FMAs with [P,1]
  runtime-coefficient APs iterated j-outer for pipelining, Horner in u_c)
  with the 7 independent G_i starter ops on ACT.

Measured engine characteristics that shaped this design:
  - ap_gather: ~115us for 32768 idxs (22-28ns/idx, Q7 software) -- THE
    bottleneck; everything else must hide under it.
  - DVE ops CONTEND ~1:1 with the concurrent gather (shared SBUF ports):
    every DVE-busy us adds ~1us of wall time; so the DVE op count is
    minimized (~44 ops) rather than the DVE critical path.
  - ACT is ~free under the gather for INDEPENDENT ops, but its exec queue
    has depth 0, so dependency-chained ACT ops pay full SBUF latency
    (~220 cycles) per hop -- chains stay on DVE.
  - PE identity-matmul psum accumulation is free under the gather but
    cross-engine ACT->PE->DVE poly pipelines measured SLOWER overall.
  - SWDGE dma_gather: ~8.6ns/idx, 1024-descriptor ring cap, Pool-serial
    -- strictly worse than ap_gather here.
  - qPoolDynamic indirect_dma_start consumes ONE offset per partition-row
    descriptor (embedding-row semantics), not per element -- unusable for
    per-element gathers.

HW exec time: 121.3us/iteration loop-slope, attempts within +-1us
(baseline 181.3us -> 1.50x), rel err 6.4e-5 (baseline 3.2e-5; gate 2e-2).
Gather-only floor measures ~117.5us, so only ~4us of non-gather time is
exposed; DEG=5 (8 fewer DVE ops) measured indistinguishable within +-3us
noise and DEG=6 is kept for out-of-distribution margin.
"""

import sys

if "/opt/trn_rl_repo" not in sys.path:
    sys.path.insert(0, "/opt/trn_rl_repo")

import numpy as np

import concourse.bass as bass
import concourse.tile as tile
from concourse import bacc, mybir
from concourse.ap import AP
from concourse.bass_utils import run_bass_kernel_spmd

F32 = mybir.dt.float32
I16 = mybir.dt.int16
U8 = mybir.dt.uint8
ALU = mybir.AluOpType

B = 262144
N_CORES = 8
BC = B // N_CORES          # 32768 per core
P = 128
FC = BC // P               # 256 free columns in natural layout
N_CONCS = 16
N_WL = 601
TBL = N_CONCS * N_WL       # 9616
MAGIC = 8388608.0          # 2^23: x + MAGIC - MAGIC == round-to-nearest-int(x)
C_MEAN, C_STD = 30.0, 30.0
WL_MEAN, WL_STD = 500.0, 300.0

DEG = 6                    # total degree of the interp polynomial
NTERMS = (DEG + 1) * (DEG + 2) // 2            # 28
USCALE = 0.125             # u = clamp(x, +-6) * 0.125  (exact pow2)
UCLAMP = 6.0 * USCALE


def coef_col(i, j):
    """Column of coefficient (i=c-degree, j=w-degree) in the s_coef tile."""
    c = 0
    for ii in range(i):
        c += DEG + 1 - ii
    return c + j


def build_nc(loop_n=1, dyn_loop=0, skip_hit=False, skip_poly=False,
             skip_gather=False, gchunks=1, gather_only=False, staggered=False):
    nc = bacc.Bacc("TRN2", target_bir_lowering=False, debug=False,
                   num_devices=N_CORES)

    # ---- dram I/O ----
    d_cn = nc.dram_tensor("cn", [P, FC], F32, kind="ExternalInput").ap()
    d_wn = nc.dram_tensor("wn", [P, FC], F32, kind="ExternalInput").ap()
    d_tbl1 = nc.dram_tensor("tbl1", [1, TBL], F32, kind="ExternalInput").ap()
    d_coef = nc.dram_tensor("coef", [P, NTERMS + 3], F32, kind="ExternalInput").ap()
    d_ident = nc.dram_tensor("ident", [P, P], F32, kind="ExternalInput").ap()
    d_out = nc.dram_tensor("out", [P, FC], F32, kind="ExternalOutput").ap()
    d_mh = nc.dram_tensor("maskh", [P, FC], U8, kind="ExternalOutput").ap()
    NJ = BC // 8                    # 4096 gather stream positions per group
    cj = NJ // gchunks
    d_exc = [
        nc.dram_tensor(f"exc{k}", [8, cj], F32, kind="ExternalOutput").ap()
        for k in range(gchunks)
    ]

    with tile.TileContext(nc) as tc:
        with (
            tc.tile_pool(name="const", bufs=1) as cpool,
            tc.tile_pool(name="hit", bufs=2) as hpool,
            tc.tile_pool(name="poly", bufs=2) as ppool,
            tc.tile_pool(name="gout", bufs=2) as gpool,
            tc.tile_pool(name="pep", bufs=1, space="PSUM") as pepool,
        ):
            s_ident = cpool.tile([P, P], F32, tag="ident")
            nc.sync.dma_start(s_ident[:], d_ident)
            s_cn = cpool.tile([P, FC], F32, tag="cn")
            nc.sync.dma_start(s_cn[:], d_cn)
            s_wn = cpool.tile([P, FC], F32, tag="wn")
            nc.sync.dma_start(s_wn[:], d_wn)
            s_coef = cpool.tile([P, NTERMS + 3], F32, tag="coef")
            nc.sync.dma_start(s_coef[:], d_coef)
            # broadcast the flat table to all 128 partitions (device side,
            # so the host->device input stays [1, TBL])
            s_tbl = cpool.tile([P, TBL], F32, tag="tbl")
            if not skip_gather:
                src = AP(d_tbl1.tensor, 0, [(0, P), (1, TBL)])
                nc.sync.dma_start(s_tbl[:], src)

            def coef(i, j):
                c = coef_col(i, j)
                return s_coef[:, c:c + 1]

            def _body():
                if gather_only:
                    g16o = hpool.tile([P, FC], I16, tag="g16o")
                    nc.vector.memset(g16o[:], 0)
                    cu = FC // gchunks
                    for k in range(gchunks):
                        dstk = gpool.tile([P, cj], F32, tag=f"dst{k}")
                        nc.gpsimd.ap_gather(
                            dstk[:], s_tbl[:], g16o[:, k * cu:(k + 1) * cu],
                            channels=P, num_elems=TBL, d=1, num_idxs=cj)
                        nc.sync.dma_start(d_exc[k], dstk[::16, :])
                    f0 = ppool.tile([P, FC], F32, tag="f")
                    nc.vector.memset(f0[0:8, 0:1], 0.0)
                    nc.sync.dma_start(d_out, f0[:])
                    nc.sync.dma_start(d_mh, g16o[:].bitcast(U8)[:, 0:FC])
                    return
                # ============== hit path (natural layout) ==============
                if not skip_hit:
                    cM = hpool.tile([P, FC], F32, tag="cM")
                    nc.vector.tensor_scalar(cM[:], s_cn[:], C_STD, C_MEAN,
                                            ALU.mult, ALU.add)
                    wM = hpool.tile([P, FC], F32, tag="wM")
                    nc.vector.tensor_scalar(wM[:], s_wn[:], WL_STD, WL_MEAN,
                                            ALU.mult, ALU.add)

                    # nearest conc index (x4): rc4 = 4*clip(round(c/4), 0, 15)
                    r1 = hpool.tile([P, FC], F32, tag="r1")
                    nc.vector.tensor_scalar(r1[:], cM[:], 0.25, MAGIC,
                                            ALU.mult, ALU.add)
                    rc = hpool.tile([P, FC], F32, tag="rc")
                    nc.vector.tensor_scalar(rc[:], r1[:], MAGIC, 0.0,
                                            ALU.subtract, ALU.max)
                    rc4 = hpool.tile([P, FC], F32, tag="rc4")
                    nc.vector.tensor_scalar(rc4[:], rc[:], 15.0, 4.0,
                                            ALU.min, ALU.mult)
                    dC = hpool.tile([P, FC], F32, tag="dC")
                    nc.vector.scalar_tensor_tensor(dC[:], rc4[:], -1.0, cM[:],
                                                   ALU.mult, ALU.add)

                    # nearest wavelength: rw6 = clip(round(wl), 200, 800) - 200
                    r1w = hpool.tile([P, FC], F32, tag="r1w")
                    nc.vector.tensor_scalar(r1w[:], wM[:], MAGIC, None, ALU.add)
                    rw = hpool.tile([P, FC], F32, tag="rw")
                    nc.vector.tensor_scalar(rw[:], r1w[:], MAGIC, 200.0,
                                            ALU.subtract, ALU.max)
                    rw6 = hpool.tile([P, FC], F32, tag="rw6")
                    nc.vector.tensor_scalar(rw6[:], rw[:], 800.0, 200.0,
                                            ALU.min, ALU.subtract)
                    ndW = hpool.tile([P, FC], F32, tag="ndW")
                    nc.vector.scalar_tensor_tensor(ndW[:], rw6[:], 200.0, wM[:],
                                                   ALU.add, ALU.subtract)

                    # flat gather index g = rc4*150.25 + rw6 (exact ints,
                    # i16 conversion fused into the op)
                    g16 = hpool.tile([P, FC], I16, tag="g16")
                    nc.vector.scalar_tensor_tensor(g16[:], rc4[:], 150.25,
                                                   rw6[:], ALU.mult, ALU.add)

                    # gather ASAP (Pool engine dominates the iteration).
                    # The exc output DMAs are issued LAST in program order:
                    # SP processes its queue in order, and an early exc DMA
                    # (waiting on the gather sem) head-of-line blocks the
                    # mask/out DMAs behind it into the post-gather tail.
                    cu = FC // gchunks
                    dsts = []
                    for k in range(gchunks):
                        dstk = gpool.tile([P, cj], F32, tag=f"dst{k}")
                        if not skip_gather:
                            nc.gpsimd.ap_gather(
                                dstk[:], s_tbl[:], g16[:, k * cu:(k + 1) * cu],
                                channels=P, num_elems=TBL, d=1, num_idxs=cj)
                        else:
                            nc.vector.memset(dstk[0:8, 0:1], 0.0)
                        dsts.append(dstk)

                    # mask = max(|dC|, |ndW|) < 0.1  (exact: both-hit iff
                    # the max is under tol)
                    aC = hpool.tile([P, FC], F32, tag="aC")
                    nc.vector.scalar_tensor_tensor(aC[:], dC[:], -1.0, dC[:],
                                                   ALU.mult, ALU.max)
                    aW = hpool.tile([P, FC], F32, tag="aW")
                    nc.vector.scalar_tensor_tensor(aW[:], ndW[:], -1.0, ndW[:],
                                                   ALU.mult, ALU.max)
                    am = hpool.tile([P, FC], F32, tag="am")
                    nc.vector.tensor_tensor(am[:], aC[:], aW[:], ALU.max)
                    mask = hpool.tile([P, FC], U8, tag="mask")
                    nc.vector.tensor_scalar(mask[:], am[:], 0.1, None, ALU.is_lt)
                    nc.sync.dma_start(d_mh, mask[:])

                # ============== interp polynomial (DVE) ==============
                f = ppool.tile([P, FC], F32, tag="f")
                if skip_poly == "dummy_dve_psum":
                    # 64 dependent-ish DVE ops entirely in PSUM
                    pp = pepool.tile([P, 2 * FC], F32, tag="dps")
                    a0 = pp[:, 0:FC]
                    a1 = pp[:, FC:2 * FC]
                    nc.vector.memset(a0, 1.0001)
                    for z in range(64):
                        nc.vector.tensor_scalar(a1 if z % 2 == 0 else a0,
                                                a0 if z % 2 == 0 else a1,
                                                1.0001, None, ALU.mult)
                    nc.vector.memset(f[:], 0.0)
                elif skip_poly == "dummy_pe":
                    # 64 f32 identity matmuls into psum under the gather
                    ps = pepool.tile([P, FC], F32, tag="pep")
                    for z in range(64):
                        nc.tensor.matmul(ps[:], s_ident[:], s_cn[:],
                                         start=(z == 0), stop=(z == 63))
                    nc.vector.memset(f[:], 0.0)
                elif skip_poly == "dummy_act2":
                    dts = []
                    for z in range(4):
                        dmt = ppool.tile([P, FC], F32, tag=f"dm{z}")
                        dts.append(dmt)
                    for z in range(128):
                        nc.scalar.mul(dts[z % 4][:], s_cn[:], 1.0001)
                    nc.vector.memset(f[:], 0.0)
                elif skip_poly == "dummy_act":
                    dts = []
                    for z in range(4):
                        dmt = ppool.tile([P, FC], F32, tag=f"dm{z}")
                        dts.append(dmt)
                    for z in range(64):
                        nc.scalar.mul(dts[z % 4][:], s_cn[:], 1.0001)
                    nc.vector.memset(f[:], 0.0)
                elif skip_poly == "dummy":
                    # pipeline-friendly independent DVE ops, same count as
                    # the real poly, no deps on hit/gather tiles
                    dts = []
                    for z in range(4):
                        dmt = ppool.tile([P, FC], F32, tag=f"dm{z}")
                        dts.append(dmt)
                    for z in range(64):
                        nc.vector.tensor_scalar(dts[z % 4][:], s_cn[:],
                                                1.0001, None, ALU.mult)
                    nc.vector.memset(f[:], 0.0)
                elif skip_poly:
                    nc.vector.memset(f[:], 0.0)
                else:
                    ACTF = mybir.ActivationFunctionType
                    b6 = s_coef[:, NTERMS:NTERMS + 1]
                    b12 = s_coef[:, NTERMS + 1:NTERMS + 2]
                    buc = s_coef[:, NTERMS + 2:NTERMS + 3]

                    # clamp+scale on DVE (2 ops/dim)
                    uc = ppool.tile([P, FC], F32, tag="uc")
                    nc.vector.tensor_scalar(uc[:], s_cn[:], USCALE, UCLAMP,
                                            ALU.mult, ALU.min)
                    nc.vector.tensor_scalar(uc[:], uc[:], -UCLAMP, None, ALU.max)
                    uw = ppool.tile([P, FC], F32, tag="uw")
                    nc.vector.tensor_scalar(uw[:], s_wn[:], USCALE, UCLAMP,
                                            ALU.mult, ALU.min)
                    nc.vector.tensor_scalar(uw[:], uw[:], -UCLAMP, None, ALU.max)

                    # power basis in u_w on DVE
                    wpow = {1: uw}
                    for j in range(2, DEG + 1):
                        t = ppool.tile([P, FC], F32, tag=f"w{j}")
                        nc.vector.tensor_tensor(t[:], wpow[j - 1][:], uw[:],
                                                ALU.mult)
                        wpow[j] = t

                    # G_i starters on ACT (independent leaf ops, ~free
                    # under the gather); FMA accumulation on DVE, iterated
                    # j-outer so consecutive DVE ops touch different G_i
                    # (pipeline-friendly)
                    G = []
                    for i in range(DEG + 1):
                        Li = DEG - i
                        g = ppool.tile([P, FC], F32, tag=f"G{i}")
                        if Li == 0:
                            nc.scalar.activation(g[:], uw[:], ACTF.Identity,
                                                 bias=coef(i, 0), scale=0.0)
                        else:
                            nc.scalar.activation(g[:], uw[:], ACTF.Identity,
                                                 bias=coef(i, 0),
                                                 scale=coef(i, 1))
                        G.append(g)
                    for j in range(2, DEG + 1):
                        for i in range(0, DEG + 1 - j):
                            nc.vector.scalar_tensor_tensor(
                                G[i][:], wpow[j][:], coef(i, j), G[i][:],
                                ALU.mult, ALU.add)

                    # Horner over u_c on DVE
                    t1 = ppool.tile([P, FC], F32, tag="ht")
                    cur = G[DEG]
                    for i in range(DEG - 1, -1, -1):
                        nc.vector.tensor_tensor(t1[:], cur[:], uc[:], ALU.mult)
                        nc.vector.tensor_tensor(f[:], t1[:], G[i][:], ALU.add)
                        cur = f
                nc.sync.dma_start(d_out, f[:])
                if not skip_hit:
                    for k in range(gchunks):
                        nc.sync.dma_start(d_exc[k], dsts[k][::16, :])

            if dyn_loop:
                with tc.For_i(0, dyn_loop, 1, staggered_reset=staggered):
                    _body()
            else:
                for _rep in range(loop_n):
                    _body()

    nc.finalize()
    return nc


_NC_CACHE = {}


def _get_nc():
    if "nc" not in _NC_CACHE:
        _NC_CACHE["nc"] = build_nc()
    return _NC_CACHE["nc"]


def fit_poly(w1, b1, w2, b2, w3, b3, w4, b4):
    """Fit the total-degree-DEG bivariate polynomial to the MLP composite
    over [-6, 6]^2 in the scaled variable u = x * USCALE. Returns [NTERMS]
    float32 coefficients (b4 folded into the constant term)."""
    w1, b1 = np.float64(w1), np.float64(b1)
    w2, b2 = np.float64(w2), np.float64(b2)
    w3, b3 = np.float64(w3), np.float64(b3)
    w4, b4 = np.float64(w4), np.float64(b4)

    ng = 48
    k = np.arange(ng)
    xg = np.cos(np.pi * (k + 0.5) / ng) * 6.0
    CG, WG = np.meshgrid(xg, xg, indexing="ij")
    x = np.stack([CG.ravel(), WG.ravel()], -1)
    h = np.tanh(x @ w1 + b1)
    h = np.tanh(h @ w2 + b2)
    h = np.tanh(h @ w3 + b3)
    fg = (h @ w4)[:, 0] + b4[0]

    uc = CG.ravel() * USCALE
    uw = WG.ravel() * USCALE
    cols = []
    for i in range(DEG + 1):
        for j in range(DEG + 1 - i):
            cols.append((uc ** i) * (uw ** j))
    A = np.stack(cols, -1)
    coefs, *_ = np.linalg.lstsq(A, fg, rcond=None)
    return coefs.astype(np.float32)


def make_in_maps(c_norm, wl_norm, train_concs, train_wavelengths, abs_matrix,
                 w1, b1, w2, b2, w3, b3, w4, b4):
    f32 = np.float32
    A = np.asarray(abs_matrix, f32)
    m = A.mean(dtype=f32).astype(f32)
    s = A.std(dtype=f32).astype(f32)
    tbl1 = ((A.ravel() - m) / s).astype(f32).reshape(1, TBL)

    coefs = fit_poly(w1, b1, w2, b2, w3, b3, w4, b4)
    coefs = np.concatenate([coefs, np.array([6.0, 12.0, UCLAMP], np.float32)])
    coef_t = np.ascontiguousarray(np.broadcast_to(coefs, (P, NTERMS + 3)))

    c_norm = np.asarray(c_norm, f32)
    wl_norm = np.asarray(wl_norm, f32)

    in_maps = []
    for i in range(N_CORES):
        sl = slice(i * BC, (i + 1) * BC)
        in_maps.append({
            "cn": c_norm[sl].reshape(P, FC),
            "wn": wl_norm[sl].reshape(P, FC),
            "tbl1": tbl1,
            "coef": coef_t,
            "ident": np.eye(P, dtype=f32),
        })
    return in_maps


def kernel(**inputs):
    nc = _get_nc()
    in_maps = make_in_maps(**inputs)
    res = run_bass_kernel_spmd(nc, in_maps, core_ids=list(range(N_CORES)))
    parts = []
    for i in range(N_CORES):
        r = res.results[i]
        interp = r["out"].astype(np.float32)                      # [P, FC]
        mask = r["maskh"] != 0                                    # [P, FC]
        # unwrap gather chunks: core-group g owns partitions 16g+r;
        # dstk[16g, 16*(j - j0) + r] = tbl[g16[16g + r, j]]
        V = np.empty((P, FC), np.float32)
        gch = len([k for k in r if k.startswith("exc")])
        cu = FC // gch
        for k in range(gch):
            E = r[f"exc{k}"].reshape(8, cu, 16)                   # [g, jc, r]
            V[:, k * cu:(k + 1) * cu] = E.transpose(0, 2, 1).reshape(P, cu)
        parts.append(np.where(mask, V, interp).ravel())
    return np.concatenate(parts).astype(np.float32)
